# revision 1
# baseline (speedup 1.0000x reference)
"""Trainium2 Bass kernel for CustomPunitiveLoss (N=8192, C=32000).

Math (identical to the reference, no max-subtraction needed since inputs
are standard normal and fp32 exp is safe for |x| < 80):
    S_i   = sum_j exp(x_ij)
    S2_i  = sum_j exp(x_ij)^2
    p_it  = exp(x_it) / S_i
    nll_i = ln(S_i) - x_it
    punish_i = (C - 2) + S2_i / S_i^2 - (1 - p_it)^2
    loss_i = nll_i + 0.1 * punish_i
    out = mean_i loss_i

Sharding: data-parallel over rows; core c gets rows [c*1024, (c+1)*1024).
Each core streams its 131 MB slice once (memory-bound), producing per-row
losses [128, 8]; the host sums and divides by N.

Engine split per [128, 4000] tile:
    sync DMA   : load tile                      (~5.7 us, bottleneck)
    ACT        : e = exp(x), accum_out = row-sum (~3.6 us)
    DVE        : fused e*e + row-sum reduce      (~4.3 us)
Target logits are fetched with 8 tiny indirect DMAs (flat offsets
row*C + target computed on host during sharding).
"""

import sys

import numpy as np

if "/opt/trn_rl_repo" not in sys.path:
    sys.path.insert(0, "/opt/trn_rl_repo")

N, C = 8192, 32000
N_CORES = 8
ROWS = N // N_CORES  # 1024 rows per core
P = 128  # SBUF partitions
RB = ROWS // P  # 8 row blocks per core
W = 4000  # column tile width
CT = C // W  # 8 column tiles
# Last row block: taper the tile widths so the post-DMA pipeline drain
# (serial ACT->DVE on the in-flight tiles) is short.
LAST_WIDTHS = [4000] * 6 + [2000] * 2 + [1000] * 4

LAST_EXEC_NS = None
LAST_RESULTS = None

_BUILT = {}


def build(rows=ROWS, c=C, w=W, last_widths=None):
    import concourse.bass as bass
    from concourse import bacc, mybir, tile

    rb = rows // P
    ct = c // w
    widths = [w] * ct
    # Graduated widths for the last row block: the serial ACT->DVE drain
    # after the final DMA lands is bounded by the last tiles' size.
    if last_widths is None:
        last_widths = widths
    assert sum(last_widths) == c, last_widths
    f32 = mybir.dt.float32
    AF = mybir.ActivationFunctionType
    OP = mybir.AluOpType
    AX = mybir.AxisListType

    nc = bacc.Bacc("TRN2", target_bir_lowering=False)
    x = nc.declare_dram_parameter("x", [rows, c], f32, isOutput=False)
    toff = nc.declare_dram_parameter("toff", [P, rb], mybir.dt.int32, isOutput=False)
    out = nc.declare_dram_parameter("out", [P, rb], f32, isOutput=True)

    with tile.TileContext(nc) as tc:
        with (
            tc.tile_pool(name="xp", bufs=4) as xp,
            tc.tile_pool(name="ep", bufs=4) as ep,
            tc.tile_pool(name="st", bufs=2) as st,
            tc.tile_pool(name="single", bufs=1) as single,
        ):
            S = single.tile([P, rb], f32)
            S2 = single.tile([P, rb], f32)

            late_act = None  # an ACT instruction from late in the loop
            for i in range(rb):
                ws = last_widths if i == rb - 1 else widths
                cti = len(ws)
                s_cols = st.tile([P, cti], f32, tag="s_cols")
                s2_cols = st.tile([P, cti], f32, tag="s2_cols")
                c0 = 0
                for j, wi in enumerate(ws):
                    x_t = xp.tile([P, wi], f32, tag="x")
                    nc.sync.dma_start(
                        out=x_t[:], in_=x[i * P : (i + 1) * P, c0 : c0 + wi]
                    )
                    c0 += wi
                    e_t = ep.tile([P, wi], f32, tag="e")
                    # e = exp(x); accum_out = per-row sum(e)
                    act = nc.scalar.activation(
                        out=e_t[:],
                        in_=x_t[:],
                        func=AF.Exp,
                        accum_out=s_cols[:, j : j + 1],
                    )
                    if i == rb - 1 and j == 0:
                        late_act = act
                    # in-place e*e with fused per-row sum: out=(e*1.0)*e,
                    # accum_out = sum(out). (tensor_tensor_reduce is not
                    # supported by this compiler/runtime; this standard
                    # TensorScalarPtr form is.)
                    nc.vector.scalar_tensor_tensor(
                        out=e_t[:],
                        in0=e_t[:],
                        scalar=1.0,
                        in1=e_t[:],
                        op0=OP.mult,
                        op1=OP.mult,
                        accum_out=s2_cols[:, j : j + 1],
                    )
                nc.vector.tensor_reduce(
                    out=S[:, i : i + 1], in_=s_cols[:], axis=AX.X, op=OP.add
                )
                nc.vector.tensor_reduce(
                    out=S2[:, i : i + 1], in_=s2_cols[:], axis=AX.X, op=OP.add
                )

            # Gather target logits x[i, t_i] via flat-offset indirect DMA.
            # Emitted AFTER the main loop so ACT's per-tile exps are not
            # ordered behind the gather-semaphore waits (they stalled the
            # whole pipeline for ~25us when emitted first). The toff load
            # goes through gpsimd (SWDGE): on sync (HWDGE) it would queue
            # FIFO behind all the x-tile DMAs and push the gathers to the
            # kernel tail; gpsimd is idle, so toff + gathers all complete
            # within the first ~25us, concurrent with the main loop.
            toff_sb = single.tile([P, rb], mybir.dt.int32)
            nc.gpsimd.dma_start(out=toff_sb[:], in_=toff[:, :])
            xt = single.tile([P, rb], f32)
            x_flat = x[:, :].rearrange("n c -> (n c)")
            for i in range(rb):
                nc.gpsimd.indirect_dma_start(
                    out=xt[:, i : i + 1],
                    out_offset=None,
                    in_=x_flat[:, None],
                    in_offset=bass.IndirectOffsetOnAxis(
                        ap=toff_sb[:, i : i + 1], axis=0
                    ),
                )

            # Final per-row math on [P, rb] (tiny).
            r = single.tile([P, rb], f32)
            nc.vector.reciprocal(out=r[:], in_=S[:])
            lnS = single.tile([P, rb], f32)
            nc.scalar.activation(out=lnS[:], in_=S[:], func=AF.Ln)
            et = single.tile([P, rb], f32)
            et_act = nc.scalar.activation(out=et[:], in_=xt[:], func=AF.Exp)
            # The scheduler otherwise hoists this tiny exp to the FRONT of
            # ACT's stream, where its wait on the gather semaphores stalls
            # every per-tile exp behind it (~16us pipeline bubble). Pin it
            # behind a late main-loop ACTIVATE (ordering-only, same engine).
            if late_act is not None:
                tile.add_dep_helper(
                    et_act.ins,
                    late_act.ins,
                    sync=False,
                    reason="keep exp(xt) out of the hot ACT stream",
                )
            pt = single.tile([P, rb], f32)
            nc.vector.tensor_tensor(out=pt[:], in0=et[:], in1=r[:], op=OP.mult)
            q = single.tile([P, rb], f32)
            nc.vector.tensor_scalar_add(out=q[:], in0=pt[:], scalar1=-1.0)
            sq = single.tile([P, rb], f32)
            nc.vector.tensor_tensor(out=sq[:], in0=q[:], in1=q[:], op=OP.mult)
            t1 = single.tile([P, rb], f32)
            nc.vector.tensor_tensor(out=t1[:], in0=S2[:], in1=r[:], op=OP.mult)
            t2 = single.tile([P, rb], f32)
            nc.vector.tensor_tensor(out=t2[:], in0=t1[:], in1=r[:], op=OP.mult)
            a = single.tile([P, rb], f32)
            nc.vector.tensor_tensor(out=a[:], in0=t2[:], in1=sq[:], op=OP.subtract)
            b = single.tile([P, rb], f32)
            nc.vector.tensor_tensor(out=b[:], in0=lnS[:], in1=xt[:], op=OP.subtract)
            # loss (without the uniform +0.1*(C-2) constant — added on host)
            lt = single.tile([P, rb], f32)
            nc.scalar.mul(out=lt[:], in_=a[:], mul=0.1)
            loss = single.tile([P, rb], f32)
            nc.vector.tensor_tensor(out=loss[:], in0=lt[:], in1=b[:], op=OP.add)
            nc.sync.dma_start(out=out[:, :], in_=loss[:])

    nc.compile()
    return nc


def _shard_inputs(x, t):
    """Per-core in_maps: x rows slice + int32 flat gather offsets [P, RB]
    with toff[p, i] = (i*P + p)*C + target[i*P + p] (local rows)."""
    in_maps = []
    rows_idx = np.arange(ROWS, dtype=np.int64)
    for core in range(N_CORES):
        r0 = core * ROWS
        flat = rows_idx * C + t[r0 : r0 + ROWS]
        toff = np.ascontiguousarray(flat.reshape(RB, P).T).astype(np.int32)
        in_maps.append({"x": x[r0 : r0 + ROWS], "toff": toff})
    return in_maps


def kernel(input, target):
    global LAST_EXEC_NS, LAST_RESULTS
    from concourse.bass_utils import run_bass_kernel_spmd

    x = np.asarray(input, dtype=np.float32)
    t = np.asarray(target).astype(np.int64).ravel()
    assert x.shape == (N, C), x.shape

    if "full" not in _BUILT:
        _BUILT["full"] = build(last_widths=LAST_WIDTHS)
    nc = _BUILT["full"]

    in_maps = _shard_inputs(x, t)
    res = run_bass_kernel_spmd(nc, in_maps, core_ids=list(range(N_CORES)))
    LAST_EXEC_NS = res.exec_time_ns
    LAST_RESULTS = res

    total = 0.0
    for core in range(N_CORES):
        total += res.results[core]["out"].astype(np.float64).sum()
    return np.float32(total / N + 0.1 * (C - 2.0))

